# revision 1
# baseline (speedup 1.0000x reference)
"""PairwiseConv1D (valid 1D conv, NWC x WIO -> NWC) on 8 TRN2 NeuronCores.

Strategy:
  - Data-parallel over batch: B=32 -> 4 batches per core, kernel replicated.
  - Host feeds x transposed per batch ([C, L]) so the contraction dim C sits
    on SBUF partitions; no on-device transposes needed.
  - out.T[f, i] = sum_k w[k].T @ xT[:, i+k] computed as 7 accumulating
    matmuls per 512-wide output chunk (PSUM bank = 512 fp32).
  - f32r matmul mode: full PE rate at N>=256 (fp32 native is 4x slower);
    measured rel err ~1.5e-4 on the 896-long contraction.
  - Raw-bass Block style with explicit semaphores: this toolchain's walrus
    codegen allows at most ONE sync-wait per instruction, so every wait is a
    standalone wait_ge on the consuming engine's queue.
  - Engines: SP = x loads (HWDGE), PE = matmuls, DVE = PSUM->SBUF copies,
    ACT = output stores (HWDGE).
"""

import numpy as np

import concourse.bass as bass
import concourse.mybir as mybir
from concourse.bass_utils import run_bass_kernel_spmd

B, L, C, K, F = 32, 8192, 128, 7, 128
NCORES = 8
BPC = B // NCORES  # batches per core
LOUT = L - K + 1  # 8186
CHUNK = 512
NCHUNK = (LOUT + CHUNK - 1) // CHUNK  # 16, last chunk = 506
NT = BPC * NCHUNK  # total psum chunks per core
NPSUM = 8  # psum banks in rotation (all of PSUM)
XDMA = 4  # DMAs per batch x-load (2 MB reads)
XCOLS = L // XDMA
GRP = 8  # output chunks per store DMA (~2 MB writes amortize R/W turnaround)
NGRPBUF = 2  # output group slots
NGRP = NCHUNK // GRP  # 2 groups per pass
ILV = 4  # chunks interleaved per weight sweep on PE

_nc = None


def _build(reps=1, detect_races=True):
    f32r = mybir.dt.float32r
    f32 = mybir.dt.float32
    nc = bass.Bass(detect_race_conditions=detect_races)
    xT = nc.dram_tensor("xT", [BPC, C, L], f32r, kind="ExternalInput")
    w = nc.dram_tensor("w", [K, C, F], f32r, kind="ExternalInput")
    outT = nc.dram_tensor("outT", [BPC, F, LOUT], f32, kind="ExternalOutput")

    G = reps * BPC  # total batch passes
    TT = G * NCHUNK  # total psum chunks

    from contextlib import ExitStack

    with ExitStack() as ctx:
        wsb = ctx.enter_context(nc.sbuf_tensor([C, K * F], f32r))
        xbuf0 = ctx.enter_context(nc.sbuf_tensor([C, L], f32r))
        xbuf1 = ctx.enter_context(nc.sbuf_tensor([C, L], f32r))
        obuf = ctx.enter_context(nc.sbuf_tensor([F, NGRPBUF * GRP * CHUNK], f32))
        psum = ctx.enter_context(nc.psum_tensor([F, NPSUM * CHUNK], f32))
        wsem = ctx.enter_context(nc.semaphore())
        # per-x-DMA-slot sems: counting one sem per slot makes waits safe
        # against out-of-order completion across HWDGE queues
        xsems = [
            ctx.enter_context(nc.semaphore(name=f"xsem{c}")) for c in range(XDMA)
        ]
        pe_sem = ctx.enter_context(nc.semaphore())
        dve_sem = ctx.enter_context(nc.semaphore())
        # per-output-group-slot sems, same reasoning
        osems = [
            ctx.enter_context(nc.semaphore(name=f"osem{s}")) for s in range(NGRPBUF)
        ]
        block = ctx.enter_context(nc.Block())

        xbufs = [xbuf0, xbuf1]

        def chunk_n(j):
            return CHUNK if j < NCHUNK - 1 else LOUT - (NCHUNK - 1) * CHUNK

        # number of x-DMA slots chunk j reads from
        def slots_needed(j):
            cols = min(L, (j + 1) * CHUNK + K - 1)
            return -(-cols // XCOLS)

        @block.sync
        def _(sync):
            # weights: [K, C, F] -> SBUF [C, (K F)]
            sync.dma_start(
                wsb[:, :], w.ap().rearrange("k c f -> c k f")
            ).then_inc(wsem, 16)
            for g in range(G):
                b = g % BPC
                if g >= 2:
                    # buffer g%2 must be fully consumed by PE (pass g-2)
                    sync.wait_ge(pe_sem, (g - 1) * NCHUNK)
                xb = xbufs[g % 2]
                for c in range(XDMA):
                    sync.dma_start(
                        xb[:, c * XCOLS : (c + 1) * XCOLS],
                        xT[b, :, c * XCOLS : (c + 1) * XCOLS],
                    ).then_inc(xsems[c], 16)
            # leave all semaphores at 0 so the NEFF can be re-executed
            QT = TT // GRP  # total output groups
            for s in range(NGRPBUF):
                sync.wait_ge(osems[s], 16 * (QT // NGRPBUF))
            for s in [wsem, pe_sem, dve_sem] + xsems + osems:
                sync.sem_clear(s)

        ilv = ILV  # chunks interleaved per weight sweep

        @block.tensor
        def _(tensor):
            tensor.wait_ge(wsem, 16)
            xseen = [0] * XDMA
            for g in range(G):
                xb = xbufs[g % 2]
                # chunk quads, k-outer within a quad: consecutive matmuls
                # share the stationary operand, easing the weight reload;
                # the other 4 PSUM banks stay free for the DVE drain
                for m in range(NCHUNK // ilv):
                    js = [ilv * m + i for i in range(ilv)]
                    ts = [g * NCHUNK + j for j in js]
                    need = 16 * (g + 1)
                    for c in range(slots_needed(js[-1])):
                        if xseen[c] < need:
                            tensor.wait_ge(xsems[c], need)
                            xseen[c] = need
                    if ts[-1] >= NPSUM:
                        tensor.wait_ge(dve_sem, ts[-1] - NPSUM + 1)
                    ns = [chunk_n(j) for j in js]
                    pss = [
                        psum[:, (t % NPSUM) * CHUNK : (t % NPSUM) * CHUNK + n]
                        for t, n in zip(ts, ns)
                    ]
                    for k in range(K):
                        for i in range(ilv):
                            ins = nc.tensor.matmul(
                                pss[i],
                                wsb[:, k * F : (k + 1) * F],
                                xb[:, js[i] * CHUNK + k : js[i] * CHUNK + k + ns[i]],
                                start=(k == 0),
                                stop=(k == K - 1),
                                skip_group_check=True,
                            )
                    ins.then_inc(pe_sem, ilv)

        @block.vector
        def _(vector):
            # drain two adjacent PSUM banks per copy (contiguous columns)
            for p in range(TT // 2):
                t = 2 * p
                j = t % NCHUNK
                n = chunk_n(j) + chunk_n(j + 1)
                q = t // GRP  # output group
                s = q % NGRPBUF  # group slot
                vector.wait_ge(pe_sem, t + 2)
                if t % GRP == 0 and q >= NGRPBUF:
                    # group slot's previous store DMA must be done
                    vector.wait_ge(osems[s], 16 * (q // NGRPBUF))
                nc.vector.tensor_copy(
                    obuf[:, s * GRP * CHUNK + (t % GRP) * CHUNK :
                         s * GRP * CHUNK + (t % GRP) * CHUNK + n],
                    psum[:, (t % NPSUM) * CHUNK : (t % NPSUM) * CHUNK + n],
                ).then_inc(dve_sem, 2)

        @block.scalar
        def _(scalar):
            QT = TT // GRP
            for q in range(QT):
                b = (q // NGRP) % BPC
                qq = q % NGRP  # group within pass
                cols0 = qq * GRP * CHUNK
                ncols = min(GRP * CHUNK, LOUT - cols0)
                s = q % NGRPBUF
                scalar.wait_ge(dve_sem, (q + 1) * GRP)
                scalar.dma_start(
                    outT[b, :, cols0 : cols0 + ncols],
                    obuf[:, s * GRP * CHUNK : s * GRP * CHUNK + ncols],
                ).then_inc(osems[s], 16)

    return nc


def kernel(x, kernel):
    global _nc
    x = np.asarray(x, dtype=np.float32)
    w = np.ascontiguousarray(np.asarray(kernel, dtype=np.float32))
    # [B, L, C, 1] -> per-batch transposed [B, C, L]
    xT = np.ascontiguousarray(np.transpose(x[..., 0], (0, 2, 1)))
    in_maps = [
        {"xT": xT[i * BPC : (i + 1) * BPC], "w": w} for i in range(NCORES)
    ]
    if _nc is None:
        _nc = _build()
    res = run_bass_kernel_spmd(_nc, in_maps, core_ids=list(range(NCORES)))
    outT = np.concatenate([r["outT"] for r in res.results], axis=0)  # [B,F,LOUT]
    out = np.transpose(outT, (0, 2, 1))[..., None]
    return np.ascontiguousarray(out).astype(np.float32)

